# revision 11
# baseline (speedup 1.0000x reference)
"""AttLSTM Trainium2 kernel: LSTM (B=256,T=2048,I=H=128) + attention pooling.

Full inputs -> full output; data-parallel over 8 NeuronCores (32 batch/core).
All-fp32 datapath: the reference's raw-score attention normalization has
denominators that nearly cancel (amplification up to ~6000x on some batch
rows), so the recurrence must track the fp32 reference trajectory closely;
bf16 anywhere in the h path fails.

Per core, one fused pass:
  scan: x-projections and bias for 16 timesteps are batched into one PSUM
    tile (4 banks): 4 fp32 matmuls (W_ih gate blocks x chunk) + per-gate
    per-partition bias added by DVE in PSUM. Each step's 4 recurrent fp32
    matmuls (W_hh @ h_{t-1}) accumulate into the step's column slice; one
    Sigmoid ACT reads the strided [u,(g,b)] slice (tanh via 2*sig(2x)-1,
    pre-scales folded into host-prepared weights; h' = h/2 with the 2x
    folded into W_hh). Cell update: 3 fused scalar_tensor_tensor DVE ops;
    h' stored fp32 to DRAM [T,H,B].
  attention: pass B streams hs to compute scores s[t,b] (DVE mul + reduce
    against partition-replicated h_last) and the denominator (ones-matmul);
    pass C re-streams hs, forms prod = hs*s, and reduces over t via a
    ones-matmul into 8 PSUM banks; final scale applies 2/den.
"""
import sys
import numpy as np

sys.path.insert(0, "/opt/trn_rl_repo")

import concourse.bass as bass  # noqa: E402
import concourse.tile as tile  # noqa: E402
from concourse import mybir  # noqa: E402
from concourse.bass_utils import run_bass_kernel_spmd  # noqa: E402
from concourse.vector_clock import ScopedClock  # noqa: E402
from contextlib import ExitStack  # noqa: E402

B, T, I, H = 256, 2048, 128, 128
NCORES = 8
BL = B // NCORES          # 32
G4 = 4 * BL               # 128 gate*batch columns

F32 = mybir.dt.float32
AF = mybir.ActivationFunctionType
OP = mybir.AluOpType
AX = mybir.AxisListType

MAX_WAITS = 1


def _split_sync_waits(nc):
    """This walrus build rejects instructions carrying more than MAX_WAITS
    semaphore waits. Hoist extras onto same-engine NoOps inserted before the
    instruction (program order on the engine queue preserves semantics)."""
    for fn in nc.m.functions:
        for bb in fn.blocks:
            new_insts = []
            for inst in bb.instructions:
                si = getattr(inst, "sync_info", None)
                waits = list(si.on_wait) if si is not None and si.on_wait else []
                if len(waits) > MAX_WAITS:
                    rest = waits[:-MAX_WAITS]
                    si.on_wait[:] = waits[-MAX_WAITS:]
                    while rest:
                        chunk, rest = rest[:MAX_WAITS], rest[MAX_WAITS:]
                        nop = mybir.InstNoOp(
                            name=f"{inst.name}-wsplit-{len(new_insts)}",
                            ins=[], outs=[],
                        )
                        nop.engine = inst.engine
                        nop.sync_info = mybir.SyncInfo(on_wait=chunk, on_update=[])
                        new_insts.append(nop)
                new_insts.append(inst)
            bb.instructions[:] = new_insts


def _patched_drain_and_barrier(self, tick_clock, wait_clock):
    nc = self.nc
    carrier = nc.sync.nop(nofuse=True)
    wait_clock.add_sem_waits(carrier.ins, ScopedClock({None: tick_clock.global_clock}))
    waits = list(carrier.ins.sync_info.on_wait)
    if len(waits) > 1:
        carrier.ins.sync_info.on_wait[:] = waits[:1]
        rest = waits[1:]
        while rest:
            nop = nc.sync.nop(nofuse=True)
            if nop.ins.sync_info is None:
                nop.ins.sync_info = mybir.SyncInfo(on_wait=list(rest[:1]), on_update=[])
            else:
                nop.ins.sync_info.on_wait[:] = rest[:1]
            rest = rest[1:]
    nc.sync.drain()
    nc.all_engine_barrier()
    assert self.sems is not None
    popped = nc._tile_sem_poison_stack.pop()
    assert popped is self._sem_poison
    nc.clear_and_free_semaphores(list(self.sems.allocated().values()))
    nc.all_engine_barrier()


tile.TileContext._drain_and_barrier = _patched_drain_and_barrier

_CACHE = {}

XCHUNK = 16               # timesteps per xp PSUM batch
SCHUNK = 128              # timesteps per attention chunk


def _build(t_steps=T, split_waits=True):
    nsc = t_steps // SCHUNK
    nc = bass.Bass()
    xT = nc.declare_dram_parameter("xT", [t_steps, I, BL], F32, isOutput=False)
    wih = nc.declare_dram_parameter("wih", [4, I, H], F32, isOutput=False)
    whh = nc.declare_dram_parameter("whh", [4, H, H], F32, isOutput=False)
    biasC = nc.declare_dram_parameter("biasC", [H, 4], F32, isOutput=False)
    ident = nc.declare_dram_parameter("ident", [H, H], F32, isOutput=False)
    ones128 = nc.declare_dram_parameter("ones128", [H, H], F32, isOutput=False)
    row1 = nc.declare_dram_parameter("row1", [1, H], F32, isOutput=False)
    out = nc.declare_dram_parameter("out", [BL, H], F32, isOutput=True)

    hs_dram = nc.dram_tensor("hs", [t_steps, H, BL], F32)
    hl_dram = nc.dram_tensor("hl", [BL, H], F32)

    with tile.TileContext(nc) as tc:
        with ExitStack() as octx:
            wpool = octx.enter_context(tc.tile_pool(name="weights", bufs=1))
            w_ih = wpool.tile([I, 4 * H], F32)
            w_hh = wpool.tile([H, 4 * H], F32)
            bias_c = wpool.tile([H, 4], F32)
            id_t = wpool.tile([H, H], F32)
            ones_t = wpool.tile([H, H], F32)
            row1_t = wpool.tile([1, H], F32)
            for g in range(4):
                nc.gpsimd.dma_start(w_ih[:, g * H:(g + 1) * H], wih[g])
                nc.gpsimd.dma_start(w_hh[:, g * H:(g + 1) * H], whh[g])
            nc.gpsimd.dma_start(bias_c[:], biasC[:])
            nc.gpsimd.dma_start(id_t[:], ident[:])
            nc.gpsimd.dma_start(ones_t[:], ones128[:])
            nc.gpsimd.dma_start(row1_t[:], row1[:])

            spool = octx.enter_context(tc.tile_pool(name="state", bufs=1))
            c_a = spool.tile([H, BL], F32)
            c_b = spool.tile([H, BL], F32)
            h_a = spool.tile([H, BL], F32)
            h_b = spool.tile([H, BL], F32)
            nc.vector.memset(c_a[:], 0.0)

            # ---------------- scan ----------------
            with ExitStack() as ctx:
                xpool = ctx.enter_context(tc.tile_pool(name="xin", bufs=3))
                gpool = ctx.enter_context(tc.tile_pool(name="gch", bufs=2, space="PSUM"))
                sigp = ctx.enter_context(tc.tile_pool(name="sig", bufs=2))
                tmpp = ctx.enter_context(tc.tile_pool(name="tmp", bufs=2))

                gch = None
                for t in range(t_steps):
                    tc_i = t % XCHUNK
                    if tc_i == 0:
                        x_sb = xpool.tile([I, XCHUNK * BL], F32, tag="x")
                        nc.gpsimd.dma_start(
                            x_sb[:].rearrange("i (t b) -> i t b", t=XCHUNK),
                            xT[t:t + XCHUNK].rearrange("t i b -> i t b"),
                        )
                        # xp chunk: [u, (g, t, b)] = 4 bank-aligned gate blocks
                        gch = gpool.tile([H, 4 * XCHUNK * BL], F32, tag="g")
                        for g in range(4):
                            nc.tensor.matmul(
                                gch[:, g * 512:(g + 1) * 512],
                                w_ih[:, g * H:(g + 1) * H],
                                x_sb[:],
                                start=True, stop=False, skip_group_check=True,
                            )
                            # bias: per-partition add, in place in PSUM
                            nc.vector.tensor_scalar_add(
                                gch[:, g * 512:(g + 1) * 512],
                                gch[:, g * 512:(g + 1) * 512],
                                bias_c[:, g:g + 1],
                            )
                    h_prev = h_a if t % 2 == 1 else h_b
                    c_prev = c_a if t % 2 == 0 else c_b
                    c_new = c_b if t % 2 == 0 else c_a
                    h_new = h_a if t % 2 == 0 else h_b

                    goff = tc_i * BL
                    if t > 0:
                        for g in range(4):
                            nc.tensor.matmul(
                                gch[:, g * 512 + goff:g * 512 + goff + BL],
                                w_hh[:, g * H:(g + 1) * H],
                                h_prev[:],
                                start=False, stop=(g == 3 and tc_i == XCHUNK - 1),
                                skip_group_check=True,
                            )
                    # sigma over [u, (g, b)] strided slice of the chunk
                    gsl = gch[:].rearrange("u (g t b) -> u t g b", g=4, t=XCHUNK)[:, tc_i]
                    sig = sigp.tile([H, G4], F32, tag="s")
                    nc.scalar.activation(
                        sig[:].rearrange("u (g b) -> u g b", g=4), gsl, AF.Sigmoid)
                    si = sig[:, 0:BL]
                    sf = sig[:, BL:2 * BL]
                    sg = sig[:, 2 * BL:3 * BL]
                    so = sig[:, 3 * BL:4 * BL]

                    u = tmpp.tile([H, BL], F32, tag="u")
                    nc.vector.scalar_tensor_tensor(u[:], sg, 0.5, si, OP.subtract, OP.mult)
                    t2 = tmpp.tile([H, BL], F32, tag="t2")
                    nc.vector.scalar_tensor_tensor(t2[:], sf, 0.0, c_prev[:], OP.bypass, OP.mult)
                    nc.vector.scalar_tensor_tensor(c_new[:], u[:], 2.0, t2[:], OP.mult, OP.add)
                    s2c = tmpp.tile([H, BL], F32, tag="s2c")
                    nc.scalar.activation(s2c[:], c_new[:], AF.Sigmoid, scale=2.0)
                    nc.vector.scalar_tensor_tensor(h_new[:], s2c[:], 0.5, so, OP.subtract, OP.mult)
                    nc.gpsimd.dma_start(hs_dram[t], h_new[:])

            h_last = h_b if (t_steps - 1) % 2 == 1 else h_a

            # ---------------- attention ----------------
            # A: replicate h'_last across partitions as [128, (b,h)]
            with ExitStack() as ctx:
                apool = octx.enter_context(tc.tile_pool(name="att", bufs=1))
                hrep = apool.tile([H, BL * H], F32)
                pp = ctx.enter_context(tc.tile_pool(name="aps", bufs=1, space="PSUM"))
                hl_ps = pp.tile([BL, H], F32, tag="tp")
                nc.tensor.transpose(hl_ps[:], h_last[:], id_t[:])
                hl_sb = apool.tile([BL, H], F32)
                nc.vector.tensor_copy(hl_sb[:], hl_ps[:])
                nc.gpsimd.dma_start(hl_dram[:], hl_sb[:])
                hl_flat = apool.tile([1, BL * H], F32)
                nc.gpsimd.dma_start(hl_flat[:], hl_dram[:].rearrange("b h -> (b h)")[None, :])
                for q in range(BL * H // 512):
                    rep_ps = pp.tile([H, 512], F32, tag="rep")
                    nc.tensor.matmul(rep_ps[:], row1_t[:],
                                     hl_flat[:, q * 512:(q + 1) * 512],
                                     start=True, stop=True)
                    nc.vector.tensor_copy(hrep[:, q * 512:(q + 1) * 512], rep_ps[:])

            # B: scores + denominator
            scpool = octx.enter_context(tc.tile_pool(name="scores", bufs=1))
            s_all = scpool.tile([SCHUNK, nsc * BL], F32)
            rden = scpool.tile([1, BL], F32)
            fin = scpool.tile([1, BL * H], F32)
            with ExitStack() as ctx:
                hspool = ctx.enter_context(tc.tile_pool(name="hs_in", bufs=2))
                p2pool = ctx.enter_context(tc.tile_pool(name="prod2", bufs=2))
                dps = ctx.enter_context(tc.tile_pool(name="den_ps", bufs=1, space="PSUM"))
                ones_f32 = scpool.tile([H, 1], F32)
                nc.vector.memset(ones_f32[:], 1.0)
                den_ps = dps.tile([1, BL], F32)
                for k in range(nsc):
                    hs_sb = hspool.tile([SCHUNK, H * BL], F32, tag="hs")
                    nc.gpsimd.dma_start(
                        hs_sb[:].rearrange("t (h b) -> t h b", h=H),
                        hs_dram[k * SCHUNK:(k + 1) * SCHUNK],
                    )
                    hs_bh = hs_sb[:].rearrange("t (h b) -> t b h", h=H)
                    prod2 = p2pool.tile([SCHUNK, BL * H], F32, tag="p2")
                    nc.vector.tensor_tensor(
                        prod2[:].rearrange("t (b h) -> t b h", b=BL),
                        hs_bh,
                        hrep[:].rearrange("t (b h) -> t b h", b=BL),
                        OP.mult)
                    s_k = s_all[:, k * BL:(k + 1) * BL]
                    nc.vector.tensor_reduce(
                        s_k, prod2[:].rearrange("t (b h) -> t b h", b=BL),
                        AX.X, OP.add)
                    nc.tensor.matmul(den_ps[:], ones_f32[:], s_k,
                                     start=(k == 0), stop=(k == nsc - 1),
                                     skip_group_check=True)
                nc.vector.reciprocal(rden[:], den_ps[:])
                nc.vector.tensor_scalar_mul(rden[:], rden[:], 2.0)

            # C: ctx accumulation (re-stream hs), then final scale
            with ExitStack() as ctx:
                hspool2 = ctx.enter_context(tc.tile_pool(name="hs_in2", bufs=2))
                prpool = ctx.enter_context(tc.tile_pool(name="prod", bufs=2))
                cxp = ctx.enter_context(tc.tile_pool(name="ctx_ps", bufs=1, space="PSUM"))
                ctx_tiles = [cxp.tile([H, 512], F32, tag=f"cx{q}", name=f"cx{q}")
                             for q in range(8)]
                for k in range(nsc):
                    hs_sb = hspool2.tile([SCHUNK, H * BL], F32, tag="hs2")
                    nc.gpsimd.dma_start(
                        hs_sb[:].rearrange("t (h b) -> t h b", h=H),
                        hs_dram[k * SCHUNK:(k + 1) * SCHUNK],
                    )
                    prod = prpool.tile([SCHUNK, BL * H], F32, tag="pr")
                    nc.vector.tensor_tensor(
                        prod[:].rearrange("t (b h) -> t b h", b=BL),
                        hs_sb[:].rearrange("t (h b) -> t b h", h=H),
                        s_all[:, k * BL:(k + 1) * BL].broadcast_to((SCHUNK, BL, H)),
                        OP.mult)
                    for q in range(8):
                        nc.tensor.matmul(
                            ctx_tiles[q][:], ones_t[:],
                            prod[:, q * 512:(q + 1) * 512],
                            start=(k == 0), stop=(k == nsc - 1),
                            skip_group_check=True,
                        )
                for q in range(8):
                    nc.vector.tensor_tensor(
                        fin[:, q * 512:(q + 1) * 512].rearrange("o (b h) -> o b h", b=4),
                        ctx_tiles[q][0:1, :].rearrange("o (b h) -> o b h", b=4),
                        rden[:, 4 * q:4 * q + 4].broadcast_to((1, 4, H)),
                        OP.mult)
                nc.gpsimd.dma_start(out[:].rearrange("b h -> (b h)")[None, :], fin[:])

    if split_waits:
        _split_sync_waits(nc)
    return nc


def _prep_consts(W_ih, W_hh, b_ih, b_hh):
    wih4 = W_ih.reshape(4, H, I).astype(np.float32).copy()
    whh4 = W_hh.reshape(4, H, H).astype(np.float32).copy()
    bias4 = (b_ih + b_hh).reshape(4, H).astype(np.float32).copy()
    wih4[2] *= 2.0
    bias4[2] *= 2.0
    whh4 *= 2.0
    whh4[2] *= 2.0
    wih_l = np.ascontiguousarray(wih4.transpose(0, 2, 1))   # [g, i, u]
    whh_l = np.ascontiguousarray(whh4.transpose(0, 2, 1))   # [g, h, u]
    biasC = np.ascontiguousarray(bias4.T)                    # [u, g]
    ident = np.eye(H, dtype=np.float32)
    ones = np.ones((H, H), np.float32)
    row1 = np.ones((1, H), np.float32)
    return wih_l, whh_l, biasC, ident, ones, row1


def kernel(x, W_ih, W_hh, b_ih, b_hh):
    if "nc" not in _CACHE:
        _CACHE["nc"] = _build()
    nc = _CACHE["nc"]
    wih_l, whh_l, biasC, ident, ones, row1 = _prep_consts(W_ih, W_hh, b_ih, b_hh)
    in_maps = []
    for c in range(NCORES):
        xs = x[c * BL:(c + 1) * BL]
        xTc = np.ascontiguousarray(xs.transpose(1, 2, 0)).astype(np.float32)
        in_maps.append({
            "xT": xTc, "wih": wih_l, "whh": whh_l, "biasC": biasC,
            "ident": ident, "ones128": ones, "row1": row1,
        })
    _CACHE["in_maps"] = in_maps
    res = run_bass_kernel_spmd(nc, in_maps, core_ids=list(range(NCORES)))
    _CACHE["last_res"] = res
    outs = [res.results[c]["out"] for c in range(NCORES)]
    return np.concatenate(outs, axis=0)[:, None, :].astype(np.float32)
